# revision 23
# baseline (speedup 1.0000x reference)
"""AdaClusteringAttention Trainium2 kernel (8 NeuronCores, batch/head parallel).

Reference semantics (per batch*head row b, cluster row = clusters[b % 8]):
  q_c/k_c/v_c = per-cluster means (segment-sum * 1/count)      [C=513, D=128]
  qk = q_c @ k_c^T ; a = softmax(qk) * counts ; a /= rowsum    [C, C]
  v  = a @ v_c ; out[n] = v[cluster[n]] ; a0 = a[:, 0]

Device strategy per core (8 rows each, all sharing ONE cluster row):
  - host: stable-sort tokens by cluster; pad each 128-cluster block's token
    list to a multiple of 128 (padding uniform across cores => one SPMD graph)
  - host packs q/k/v for the core's 8 rows token-major into one bf16 tensor
    qkv8[n, (tensor,row,d)] so a single dma_gather descriptor moves 6KB per
    token (Q7 descriptor generation is the scarce resource)
  - segment sums = per-128-token-chunk matmuls against one-hot blocks
    (a chunk's tokens all fall inside one 128-cluster block); row pairs are
    adjacent in the gathered layout so matmuls run at N=256
  - cluster attention: qkT = k_c q_c^T, aT = exp(qkT + ln(count[e]))
    (count-weighted softmax; max-subtraction skipped, scale cancels)
  - v_out = aT^T @ [v_c | 1] gives numerator and rowsum together
  - out tokens via one-hot-transpose matmuls in sorted order into a
    row-grouped buffer, then dma_scatter_add (4KB/token) back to token
    order (outputs are zero-initialized; pads go to dump row NSEQ)
"""

import sys

import numpy as np

B0, H, NSEQ, D = 8, 8, 4096, 128
B = B0 * H
C = 513
NBLK = 5            # ceil(C/128) cluster blocks
CPAD = NBLK * 128   # 640
NCORES = 8
ROWS = B // NCORES  # 8 rows per core
LN_NEG = -88.0      # exp(-88) == 0 in f32/bf16
OPTOK = 512         # tokens per gather/scatter op (SBUF footprint knob)


def _bf16():
    import ml_dtypes
    return ml_dtypes.bfloat16


# ----------------------------------------------------------------- host meta

class Meta:
    pass


def build_meta(clusters: np.ndarray) -> Meta:
    m = Meta()
    assert clusters.shape == (B0, NSEQ)
    counts = np.zeros((B0, CPAD), np.int64)
    for i in range(B0):
        counts[i, :C] = np.bincount(clusters[i], minlength=C)
    blk_tok = counts.reshape(B0, NBLK, 128).sum(-1)
    T_m = np.maximum(128, (np.ceil(blk_tok.max(0) / 128) * 128).astype(np.int64))
    m.T_m = tuple(int(x) for x in T_m)
    m.TT = int(T_m.sum())
    m.Tc = m.TT // 128
    offs = np.concatenate([[0], np.cumsum(T_m)]).astype(np.int64)
    m.offs = offs
    blk_of_chunk = []
    for mm in range(NBLK):
        blk_of_chunk += [mm] * (m.T_m[mm] // 128)
    m.blk_of_chunk = tuple(blk_of_chunk)
    first, last = {}, {}
    for t, mm in enumerate(m.blk_of_chunk):
        first.setdefault(mm, t)
        last[mm] = t
    m.first_chunk = first
    m.last_chunk = last

    bf16 = _bf16()
    m.cores = []
    for i in range(B0):
        cm = Meta()
        cl = clusters[i].astype(np.int64)
        order = np.argsort(cl, kind="stable")
        sc = cl[order]
        sblk = sc // 128
        idx_g = np.zeros(m.TT, np.int64)           # gather pad -> token 0
        idx_s = np.full(m.TT, NSEQ, np.int64)      # scatter pad -> dump row
        P = np.zeros((m.TT, 128), np.float32)
        for mm in range(NBLK):
            lo = int(np.searchsorted(sblk, mm))
            hi = int(np.searchsorted(sblk, mm + 1))
            if hi == lo:
                continue
            dst = offs[mm] + np.arange(hi - lo)
            idx_g[dst] = order[lo:hi]
            idx_s[dst] = order[lo:hi]
            P[dst, sc[lo:hi] - 128 * mm] = 1.0
        cm.P_sb = np.ascontiguousarray(
            P.reshape(m.Tc, 128, 128).transpose(1, 0, 2).reshape(128, m.Tc * 128)
        ).astype(bf16)
        cm.PT_sb = np.ascontiguousarray(
            P.reshape(m.Tc, 128, 128).transpose(2, 0, 1).reshape(128, m.Tc * 128)
        ).astype(bf16)
        cm.idx_g = np.ascontiguousarray(np.tile(
            idx_g.reshape(m.TT // 16, 16).T, (8, 1))).astype(np.int16)
        cm.idx_s = np.ascontiguousarray(np.tile(
            idx_s.reshape(m.TT // 16, 16).T, (8, 1))).astype(np.int16)
        cnts = counts[i].astype(np.float64)
        w = np.where(cnts > 0, 1.0 / np.maximum(cnts, 1), 0.0)
        lnc = np.where(cnts > 0, np.log(np.maximum(cnts, 1)), LN_NEG)
        cm.wcol = np.ascontiguousarray(
            w.reshape(NBLK, 128).T).astype(np.float32)
        cm.lncnt = np.ascontiguousarray(
            lnc.reshape(NBLK, 128).T).astype(np.float32)
        m.cores.append(cm)
    return m


def out_groups(n_rows):
    HR = max(n_rows // 2, 1)
    groups = [(0, HR)]
    if n_rows > HR:
        groups.append((HR, n_rows - HR))
    return groups


# ------------------------------------------------------------- bass builder

def build_nc(meta: Meta, n_rows: int = ROWS):
    import concourse.bacc as bacc
    import concourse.mybir as mybir
    import concourse.tile as tile
    from concourse import bass
    from concourse.masks import make_identity

    dt = mybir.dt
    Tc, TT, BLK = meta.Tc, meta.TT, meta.blk_of_chunk
    EW = 3 * n_rows * D           # gathered row width (elems, bf16)
    OW = n_rows * D               # out8 row width (elems, f32)
    n_ops = (TT + OPTOK - 1) // OPTOK
    op_tok = [min(OPTOK, TT - c * OPTOK) for c in range(n_ops)]

    nc = bacc.Bacc("TRN2", target_bir_lowering=False, debug=False,
                   num_devices=NCORES)

    qkv_ext = nc.dram_tensor("qkv", [NSEQ, EW], dt.bfloat16, kind="ExternalInput")
    P_ext = nc.dram_tensor("P", [128, Tc * 128], dt.bfloat16, kind="ExternalInput")
    PT_ext = nc.dram_tensor("PT", [128, Tc * 128], dt.bfloat16, kind="ExternalInput")
    ig_ext = nc.dram_tensor("idxg", [128, TT // 16], dt.int16, kind="ExternalInput")
    is_ext = nc.dram_tensor("idxs", [128, TT // 16], dt.int16, kind="ExternalInput")
    w_ext = nc.dram_tensor("wcol", [128, NBLK], dt.float32, kind="ExternalInput")
    ln_ext = nc.dram_tensor("lncnt", [128, NBLK], dt.float32, kind="ExternalInput")
    groups = out_groups(n_rows)
    out_exts = [
        nc.dram_tensor(f"out{gi}", [NSEQ + 1, nr * D], dt.bfloat16,
                       kind="ExternalOutput")
        for gi, (rlo, nr) in enumerate(groups)
    ]
    a0_ext = nc.dram_tensor("a0T", [128, NBLK * n_rows], dt.float32,
                            kind="ExternalOutput")

    Exp = mybir.ActivationFunctionType.Exp
    QW = min(4, n_rows)           # rows per segsum matmul (one PSUM bank)
    nquad = n_rows // QW

    with tile.TileContext(nc) as tc:
        with (
            tc.tile_pool(name="const", bufs=1) as constp,
            tc.tile_pool(name="gath", bufs=2) as gathp,
            tc.tile_pool(name="rowbuf", bufs=3) as rowp,
            tc.tile_pool(name="persist", bufs=1) as perp,
            tc.tile_pool(name="osort", bufs=3) as osortp,
            tc.tile_pool(name="small", bufs=4) as smallp,
        ):
            # ---- constants
            P_sb = constp.tile([128, Tc * 128], dt.bfloat16)
            PT_sb = constp.tile([128, Tc * 128], dt.bfloat16)
            ig_sb = constp.tile([128, TT // 16], dt.int16)
            is_sb = constp.tile([128, TT // 16], dt.int16)
            w_sb = constp.tile([128, NBLK], dt.float32)
            ln_sb = constp.tile([128, NBLK], dt.float32)
            ident_sb = constp.tile([128, 128], dt.bfloat16)
            nc.sync.dma_start(ig_sb[:, :], ig_ext[:, :])
            nc.sync.dma_start(is_sb[:, :], is_ext[:, :])
            nc.sync.dma_start(w_sb[:, :], w_ext[:, :])
            nc.sync.dma_start(ln_sb[:, :], ln_ext[:, :])
            nc.sync.dma_start(P_sb[:, :], P_ext[:, :])
            nc.sync.dma_start(PT_sb[:, :], PT_ext[:, :])
            make_identity(nc, ident_sb[:, :])

            # ---- persistent per-row results
            qc_cd = perp.tile([128, n_rows, NBLK, D], dt.bfloat16)
            kc_cd = perp.tile([128, n_rows, NBLK, D], dt.bfloat16)
            v_aug = perp.tile([128, n_rows, NBLK, D + 4], dt.bfloat16)
            v_nrm = perp.tile([128, n_rows, NBLK, D], dt.bfloat16)
            a0_all = perp.tile([128, NBLK, n_rows], dt.float32)
            qdc = perp.tile([128, n_rows, NBLK, 128], dt.bfloat16)
            kdc = perp.tile([128, n_rows, NBLK, 128], dt.bfloat16)
            nc.vector.memset(a0_all[:, :, :], 0.0)
            nc.vector.memset(v_aug[:, :, :, D:D + 1], 1.0)
            # e0 column: picks out aT[0, :] inside the av matmul (e-chunk 0)
            nc.vector.memset(v_aug[:, :, :, D + 1:D + 2], 0.0)
            nc.vector.memset(v_aug[0:1, :, 0:1, D + 1:D + 2], 1.0)

            # ---- phase 1: gather + segment sums for all rows at once
            ph1 = tc.tile_pool(name="pscd", bufs=1, space="PSUM")
            pscdp = ph1.__enter__()
            ph1t = tc.tile_pool(name="pst", bufs=2, space="PSUM")
            pstp = ph1t.__enter__()
            ps_cd = {}
            for x in range(3):
                ps_cd[x] = pscdp.tile([128, n_rows * D], dt.float32,
                                      tag=f"cd{x}", name=f"ps_cd{x}")
            gtiles = []
            for c in range(n_ops):
                g = gathp.tile([128, OPTOK // 128, EW], dt.bfloat16, tag="gath")
                nc.gpsimd.dma_gather(
                    out_ap=g[:, 0:op_tok[c] // 128, :],
                    in_ap=qkv_ext[:, :],
                    idxs_ap=ig_sb[:, c * (OPTOK // 16):
                                  c * (OPTOK // 16) + op_tok[c] // 16],
                    num_idxs=op_tok[c],
                    num_idxs_reg=op_tok[c],
                    elem_size=EW,
                    single_packet=False,
                )
                gtiles.append(g)

            for t in range(Tc):
                mm = BLK[t]
                c, t_loc = t // (OPTOK // 128), t % (OPTOK // 128)
                g = gtiles[c]
                for x in range(3):
                    for q in range(nquad):
                        nc.tensor.matmul(
                            ps_cd[x][:, bass.ts(q, QW * D)],
                            lhsT=P_sb[:, bass.ts(t, 128)],
                            rhs=g[:, t_loc,
                                  (x * n_rows + QW * q) * D:
                                  (x * n_rows + QW * (q + 1)) * D],
                            start=(t == meta.first_chunk[mm]),
                            stop=(t == meta.last_chunk[mm]),
                        )
                if t == meta.last_chunk[mm]:
                    for x, dst in ((0, qc_cd), (1, kc_cd), (2, v_aug)):
                        nc.vector.tensor_scalar_mul(
                            dst[:, :, mm, 0:D], ps_cd[x][:, :],
                            w_sb[:, mm:mm + 1])
                    for src_t, dst_t in ((qc_cd, qdc), (kc_cd, kdc)):
                        for r in range(n_rows):
                            pst = pstp.tile([128, 128], dt.bfloat16,
                                            tag="pst", name=f"pst{mm}_{r}")
                            nc.tensor.transpose(pst[:, :],
                                                src_t[:, r, mm, 0:D],
                                                ident_sb[:, :])
                            nc.vector.tensor_copy(dst_t[:, r, mm, :],
                                                  pst[:, :])

            ph1t.__exit__(None, None, None)
            ph1.__exit__(None, None, None)

            # ---- phase 2/3 PSUM pools
            ph2a = tc.tile_pool(name="psqk", bufs=3, space="PSUM")
            psqkp = ph2a.__enter__()
            ph2b = tc.tile_pool(name="psmall", bufs=5, space="PSUM")
            psmallp = ph2b.__enter__()

            # ---- phase 2+3 interleaved by row halves
            def phase2(r):
                qdc_f = qdc[:, r, :, :].rearrange("p a b -> p (a b)")
                aT = rowp.tile([128, NBLK, 520], dt.bfloat16, tag="aT",
                               name=f"aT{r}")
                for j in range(NBLK):
                    psA = psqkp.tile([128, 512], dt.float32, tag="psqk")
                    psB = psmallp.tile([128, 8], dt.float32, tag="psmall")
                    nc.tensor.matmul(psA[:, :], lhsT=kdc[:, r, j, :],
                                     rhs=qdc_f[:, 0:512], start=True, stop=True)
                    nc.tensor.matmul(psB[:, :], lhsT=kdc[:, r, j, :],
                                     rhs=qdc_f[:, 512:520], start=True, stop=True)
                    nc.scalar.activation(aT[:, j, 0:512], psA[:, :], Exp,
                                         bias=ln_sb[:, j:j + 1])
                    nc.scalar.activation(aT[:, j, 512:520], psB[:, :], Exp,
                                         bias=ln_sb[:, j:j + 1])

                for i in range(NBLK):
                    M = 128 if i < NBLK - 1 else (C - 128 * (NBLK - 1))
                    psv = psmallp.tile([M, D + 4], dt.float32, tag="psmall")
                    for j in range(NBLK):
                        nc.tensor.matmul(
                            psv[:, 0:D + 2],
                            lhsT=aT[:, j, 128 * i:128 * i + M],
                            rhs=v_aug[:, r, j, 0:D + 2],
                            start=(j == 0), stop=(j == NBLK - 1),
                        )
                    rv = smallp.tile([M, 1], dt.float32, tag="rv")
                    nc.vector.reciprocal(rv[:, :], psv[:, D:D + 1])
                    if M < 128:
                        nc.vector.memset(v_nrm[:, r, i, :], 0.0)
                    nc.vector.tensor_scalar_mul(v_nrm[0:M, r, i, :],
                                                psv[:, 0:D], rv[:, :])
                    nc.vector.tensor_mul(a0_all[0:M, i, r:r + 1],
                                         psv[:, D + 1:D + 2], rv[:, :])

            SCTOK = 512
            sc_ops = (TT + SCTOK - 1) // SCTOK
            sc_tok = [min(SCTOK, TT - c * SCTOK) for c in range(sc_ops)]

            def phase3(half, rlo, nr, ext):
                hw_ = nr * D
                for c in range(sc_ops):
                    ntok = sc_tok[c]
                    osort = osortp.tile([128, SCTOK // 128, hw_], dt.bfloat16,
                                        tag=f"osort{half}",
                                        name=f"osort{half}_{c}")
                    for t_loc in range(ntok // 128):
                        t = c * (SCTOK // 128) + t_loc
                        pso = psmallp.tile([128, hw_], dt.float32,
                                           tag="psmall", name=f"pso{half}_{t}")
                        nc.tensor.matmul(
                            pso[:, :],
                            lhsT=PT_sb[:, bass.ts(t, 128)],
                            rhs=v_nrm[:, rlo:rlo + nr, BLK[t], :],
                            start=True, stop=True)
                        if t_loc % 2:
                            nc.scalar.copy(osort[:, t_loc, :], pso[:, :])
                        else:
                            nc.vector.tensor_copy(osort[:, t_loc, :], pso[:, :])
                    nc.gpsimd.dma_scatter_add(
                        ext[:, :],
                        osort[:, 0:ntok // 128, :],
                        is_sb[:, c * (SCTOK // 16):
                              c * (SCTOK // 16) + ntok // 16],
                        ntok,
                        ntok,
                        hw_,
                        single_packet=False,
                    )

            for gi, (rlo, nr) in enumerate(groups):
                for r in range(rlo, rlo + nr):
                    phase2(r)
                phase3(gi, rlo, nr, out_exts[gi])
            nc.sync.dma_start(a0_ext[:, :],
                              a0_all[:, :, :].rearrange("p a b -> p (a b)"))
            ph2b.__exit__(None, None, None)
            ph2a.__exit__(None, None, None)

    return nc


# ------------------------------------------------------------------- runner

_CACHE = {}


def _ensure_ntff_hook():
    import types
    if "antenv.axon_hooks" in sys.modules:
        return
    m = types.ModuleType("antenv.axon_hooks")
    m._hook = None
    m.set_axon_ntff_profile_hook = lambda h: setattr(m, "_hook", h)
    m.get_axon_ntff_profile_hook = lambda: m._hook
    sys.modules["antenv.axon_hooks"] = m
    try:
        import antenv
        antenv.axon_hooks = m
    except Exception:
        pass
    try:
        from trn_agent_boot.trn_boot import _ntff_profile_via_ctypes
        hook = _ntff_profile_via_ctypes("/opt/axon/libaxon_pjrt.so")
        if hook is not None:
            m._hook = hook
    except Exception:
        pass


def make_in_maps(queries, keys, values, meta, n_rows=ROWS):
    bf16 = _bf16()
    in_maps = []
    for i in range(NCORES):
        cm = meta.cores[i]
        # token-major pack: qkv8[n, x*n_rows + r, :] = X_x[i + 8r, n, :]
        qkv = np.empty((NSEQ, 3, n_rows, D), dtype=bf16)
        for x, src in enumerate((queries, keys, values)):
            rows = src[i::NCORES][:n_rows]          # [n_rows, NSEQ, D]
            qkv[:, x, :, :] = rows.transpose(1, 0, 2).astype(bf16)
        in_maps.append({
            "qkv": qkv.reshape(NSEQ, 3 * n_rows * D),
            "P": cm.P_sb,
            "PT": cm.PT_sb,
            "idxg": cm.idx_g,
            "idxs": cm.idx_s,
            "wcol": cm.wcol,
            "lncnt": cm.lncnt,
        })
    return in_maps


def run_cores(queries, keys, values, clusters, trace=False, n_rows=ROWS):
    _ensure_ntff_hook()
    from concourse.bass_utils import run_bass_kernel_spmd

    meta = build_meta(np.asarray(clusters))
    key = (meta.T_m, n_rows)
    if key not in _CACHE:
        nc = build_nc(meta, n_rows)
        nc.finalize()
        _CACHE[key] = nc
    nc = _CACHE[key]
    in_maps = make_in_maps(np.asarray(queries), np.asarray(keys),
                           np.asarray(values), meta, n_rows)
    res = run_bass_kernel_spmd(nc, in_maps, core_ids=list(range(NCORES)),
                               trace=trace)
    return res, meta


def unshard(res, n_rows=ROWS):
    out = np.empty((B, NSEQ, D), np.float32)
    a0 = np.empty((B, C), np.float32)
    for i in range(NCORES):
        r = res.results[i]
        for gi, (rlo, nr) in enumerate(out_groups(n_rows)):
            og = np.asarray(r[f"out{gi}"][:NSEQ], np.float32).reshape(
                NSEQ, nr, D)
            for rr in range(nr):
                out[i + NCORES * (rlo + rr)] = og[:, rr, :]
        a0T = r["a0T"].reshape(128, NBLK, n_rows)
        for rr in range(n_rows):
            a0[i + NCORES * rr] = a0T[:, :, rr].T.reshape(CPAD)[:C]
    return out, a0


def kernel(**inputs):
    queries = np.asarray(inputs["queries"], np.float32)
    keys = np.asarray(inputs["keys"], np.float32)
    values = np.asarray(inputs["values"], np.float32)
    clusters = np.asarray(inputs["clusters"], np.int32)
    res, _ = run_cores(queries, keys, values, clusters, trace=False)
    return unshard(res)


# revision 24
# speedup vs baseline: 1.0075x; 1.0075x over previous
"""AdaClusteringAttention Trainium2 kernel (8 NeuronCores, batch/head parallel).

Reference semantics (per batch*head row b, cluster row = clusters[b % 8]):
  q_c/k_c/v_c = per-cluster means (segment-sum * 1/count)      [C=513, D=128]
  qk = q_c @ k_c^T ; a = softmax(qk) * counts ; a /= rowsum    [C, C]
  v  = a @ v_c ; out[n] = v[cluster[n]] ; a0 = a[:, 0]

Device strategy per core (8 rows each, all sharing ONE cluster row):
  - host: stable-sort tokens by cluster; pad each 128-cluster block's token
    list to a multiple of 128 (padding uniform across cores => one SPMD graph)
  - host packs q/k/v for the core's 8 rows token-major into one bf16 tensor
    qkv8[n, (tensor,row,d)] so a single dma_gather descriptor moves 6KB per
    token (Q7 descriptor generation is the scarce resource)
  - segment sums = per-128-token-chunk matmuls against one-hot blocks
    (a chunk's tokens all fall inside one 128-cluster block); row pairs are
    adjacent in the gathered layout so matmuls run at N=256
  - cluster attention: qkT = k_c q_c^T, aT = exp(qkT + ln(count[e]))
    (count-weighted softmax; max-subtraction skipped, scale cancels)
  - v_out = aT^T @ [v_c | 1] gives numerator and rowsum together
  - out tokens via one-hot-transpose matmuls in sorted order into a
    row-grouped buffer, then dma_scatter_add (4KB/token) back to token
    order (outputs are zero-initialized; pads go to dump row NSEQ)
"""

import sys

import numpy as np

B0, H, NSEQ, D = 8, 8, 4096, 128
B = B0 * H
C = 513
NBLK = 5            # ceil(C/128) cluster blocks
CPAD = NBLK * 128   # 640
NCORES = 8
ROWS = B // NCORES  # 8 rows per core
LN_NEG = -88.0      # exp(-88) == 0 in f32/bf16
OPTOK = 512         # tokens per gather/scatter op (SBUF footprint knob)


def _bf16():
    import ml_dtypes
    return ml_dtypes.bfloat16


# ----------------------------------------------------------------- host meta

class Meta:
    pass


def build_meta(clusters: np.ndarray) -> Meta:
    m = Meta()
    assert clusters.shape == (B0, NSEQ)
    counts = np.zeros((B0, CPAD), np.int64)
    for i in range(B0):
        counts[i, :C] = np.bincount(clusters[i], minlength=C)
    blk_tok = counts.reshape(B0, NBLK, 128).sum(-1)
    T_m = np.maximum(128, (np.ceil(blk_tok.max(0) / 128) * 128).astype(np.int64))
    m.T_m = tuple(int(x) for x in T_m)
    m.TT = int(T_m.sum())
    m.Tc = m.TT // 128
    offs = np.concatenate([[0], np.cumsum(T_m)]).astype(np.int64)
    m.offs = offs
    blk_of_chunk = []
    for mm in range(NBLK):
        blk_of_chunk += [mm] * (m.T_m[mm] // 128)
    m.blk_of_chunk = tuple(blk_of_chunk)
    first, last = {}, {}
    for t, mm in enumerate(m.blk_of_chunk):
        first.setdefault(mm, t)
        last[mm] = t
    m.first_chunk = first
    m.last_chunk = last

    bf16 = _bf16()
    m.cores = []
    for i in range(B0):
        cm = Meta()
        cl = clusters[i].astype(np.int64)
        order = np.argsort(cl, kind="stable")
        sc = cl[order]
        sblk = sc // 128
        idx_g = np.zeros(m.TT, np.int64)           # gather pad -> token 0
        idx_s = np.full(m.TT, NSEQ, np.int64)      # scatter pad -> dump row
        P = np.zeros((m.TT, 128), np.float32)
        for mm in range(NBLK):
            lo = int(np.searchsorted(sblk, mm))
            hi = int(np.searchsorted(sblk, mm + 1))
            if hi == lo:
                continue
            dst = offs[mm] + np.arange(hi - lo)
            idx_g[dst] = order[lo:hi]
            idx_s[dst] = order[lo:hi]
            P[dst, sc[lo:hi] - 128 * mm] = 1.0
        cm.P_sb = np.ascontiguousarray(
            P.reshape(m.Tc, 128, 128).transpose(1, 0, 2).reshape(128, m.Tc * 128)
        ).astype(bf16)
        cm.PT_sb = np.ascontiguousarray(
            P.reshape(m.Tc, 128, 128).transpose(2, 0, 1).reshape(128, m.Tc * 128)
        ).astype(bf16)
        cm.idx_g = np.ascontiguousarray(np.tile(
            idx_g.reshape(m.TT // 16, 16).T, (8, 1))).astype(np.int16)
        cm.idx_s = np.ascontiguousarray(np.tile(
            idx_s.reshape(m.TT // 16, 16).T, (8, 1))).astype(np.int16)
        cnts = counts[i].astype(np.float64)
        w = np.where(cnts > 0, 1.0 / np.maximum(cnts, 1), 0.0)
        lnc = np.where(cnts > 0, np.log(np.maximum(cnts, 1)), LN_NEG)
        cm.wcol = np.ascontiguousarray(
            w.reshape(NBLK, 128).T).astype(np.float32)
        cm.lncnt = np.ascontiguousarray(
            lnc.reshape(NBLK, 128).T).astype(np.float32)
        m.cores.append(cm)
    return m


def out_groups(n_rows):
    HR = max(n_rows // 2, 1)
    groups = [(0, HR)]
    if n_rows > HR:
        groups.append((HR, n_rows - HR))
    return groups


# ------------------------------------------------------------- bass builder

def build_nc(meta: Meta, n_rows: int = ROWS):
    import concourse.bacc as bacc
    import concourse.mybir as mybir
    import concourse.tile as tile
    from concourse import bass
    from concourse.masks import make_identity

    dt = mybir.dt
    Tc, TT, BLK = meta.Tc, meta.TT, meta.blk_of_chunk
    EW = 3 * n_rows * D           # gathered row width (elems, bf16)
    OW = n_rows * D               # out8 row width (elems, f32)
    n_ops = (TT + OPTOK - 1) // OPTOK
    op_tok = [min(OPTOK, TT - c * OPTOK) for c in range(n_ops)]

    nc = bacc.Bacc("TRN2", target_bir_lowering=False, debug=False,
                   num_devices=NCORES, num_swdge_queues=2)

    qkv_ext = nc.dram_tensor("qkv", [NSEQ, EW], dt.bfloat16, kind="ExternalInput")
    P_ext = nc.dram_tensor("P", [128, Tc * 128], dt.bfloat16, kind="ExternalInput")
    PT_ext = nc.dram_tensor("PT", [128, Tc * 128], dt.bfloat16, kind="ExternalInput")
    ig_ext = nc.dram_tensor("idxg", [128, TT // 16], dt.int16, kind="ExternalInput")
    is_ext = nc.dram_tensor("idxs", [128, TT // 16], dt.int16, kind="ExternalInput")
    w_ext = nc.dram_tensor("wcol", [128, NBLK], dt.float32, kind="ExternalInput")
    ln_ext = nc.dram_tensor("lncnt", [128, NBLK], dt.float32, kind="ExternalInput")
    groups = out_groups(n_rows)
    out_exts = [
        nc.dram_tensor(f"out{gi}", [NSEQ + 1, nr * D], dt.bfloat16,
                       kind="ExternalOutput")
        for gi, (rlo, nr) in enumerate(groups)
    ]
    a0_ext = nc.dram_tensor("a0T", [128, NBLK * n_rows], dt.float32,
                            kind="ExternalOutput")

    Exp = mybir.ActivationFunctionType.Exp
    QW = min(4, n_rows)           # rows per segsum matmul (one PSUM bank)
    nquad = n_rows // QW

    with tile.TileContext(nc) as tc:
        with (
            tc.tile_pool(name="const", bufs=1) as constp,
            tc.tile_pool(name="gath", bufs=2) as gathp,
            tc.tile_pool(name="rowbuf", bufs=3) as rowp,
            tc.tile_pool(name="persist", bufs=1) as perp,
            tc.tile_pool(name="osort", bufs=3) as osortp,
            tc.tile_pool(name="small", bufs=4) as smallp,
        ):
            # ---- constants
            P_sb = constp.tile([128, Tc * 128], dt.bfloat16)
            PT_sb = constp.tile([128, Tc * 128], dt.bfloat16)
            ig_sb = constp.tile([128, TT // 16], dt.int16)
            is_sb = constp.tile([128, TT // 16], dt.int16)
            w_sb = constp.tile([128, NBLK], dt.float32)
            ln_sb = constp.tile([128, NBLK], dt.float32)
            ident_sb = constp.tile([128, 128], dt.bfloat16)
            nc.sync.dma_start(ig_sb[:, :], ig_ext[:, :])
            nc.sync.dma_start(is_sb[:, :], is_ext[:, :])
            nc.sync.dma_start(w_sb[:, :], w_ext[:, :])
            nc.sync.dma_start(ln_sb[:, :], ln_ext[:, :])
            nc.sync.dma_start(P_sb[:, :], P_ext[:, :])
            nc.sync.dma_start(PT_sb[:, :], PT_ext[:, :])
            make_identity(nc, ident_sb[:, :])

            # ---- persistent per-row results
            qc_cd = perp.tile([128, n_rows, NBLK, D], dt.bfloat16)
            kc_cd = perp.tile([128, n_rows, NBLK, D], dt.bfloat16)
            v_aug = perp.tile([128, n_rows, NBLK, D + 4], dt.bfloat16)
            v_nrm = perp.tile([128, n_rows, NBLK, D], dt.bfloat16)
            a0_all = perp.tile([128, NBLK, n_rows], dt.float32)
            qdc = perp.tile([128, n_rows, NBLK, 128], dt.bfloat16)
            kdc = perp.tile([128, n_rows, NBLK, 128], dt.bfloat16)
            nc.vector.memset(a0_all[:, :, :], 0.0)
            nc.vector.memset(v_aug[:, :, :, D:D + 1], 1.0)
            # e0 column: picks out aT[0, :] inside the av matmul (e-chunk 0)
            nc.vector.memset(v_aug[:, :, :, D + 1:D + 2], 0.0)
            nc.vector.memset(v_aug[0:1, :, 0:1, D + 1:D + 2], 1.0)

            # ---- phase 1: gather + segment sums for all rows at once
            ph1 = tc.tile_pool(name="pscd", bufs=1, space="PSUM")
            pscdp = ph1.__enter__()
            ph1t = tc.tile_pool(name="pst", bufs=2, space="PSUM")
            pstp = ph1t.__enter__()
            ps_cd = {}
            for x in range(3):
                ps_cd[x] = pscdp.tile([128, n_rows * D], dt.float32,
                                      tag=f"cd{x}", name=f"ps_cd{x}")
            gtiles = []
            for c in range(n_ops):
                g = gathp.tile([128, OPTOK // 128, EW], dt.bfloat16, tag="gath")
                nc.gpsimd.dma_gather(
                    out_ap=g[:, 0:op_tok[c] // 128, :],
                    in_ap=qkv_ext[:, :],
                    idxs_ap=ig_sb[:, c * (OPTOK // 16):
                                  c * (OPTOK // 16) + op_tok[c] // 16],
                    num_idxs=op_tok[c],
                    num_idxs_reg=op_tok[c],
                    elem_size=EW,
                    single_packet=False,
                    queue_num=c % 2,
                )
                gtiles.append(g)

            for t in range(Tc):
                mm = BLK[t]
                c, t_loc = t // (OPTOK // 128), t % (OPTOK // 128)
                g = gtiles[c]
                for x in range(3):
                    for q in range(nquad):
                        nc.tensor.matmul(
                            ps_cd[x][:, bass.ts(q, QW * D)],
                            lhsT=P_sb[:, bass.ts(t, 128)],
                            rhs=g[:, t_loc,
                                  (x * n_rows + QW * q) * D:
                                  (x * n_rows + QW * (q + 1)) * D],
                            start=(t == meta.first_chunk[mm]),
                            stop=(t == meta.last_chunk[mm]),
                        )
                if t == meta.last_chunk[mm]:
                    for x, dst in ((0, qc_cd), (1, kc_cd), (2, v_aug)):
                        nc.vector.tensor_scalar_mul(
                            dst[:, :, mm, 0:D], ps_cd[x][:, :],
                            w_sb[:, mm:mm + 1])
                    for src_t, dst_t in ((qc_cd, qdc), (kc_cd, kdc)):
                        for r in range(n_rows):
                            pst = pstp.tile([128, 128], dt.bfloat16,
                                            tag="pst", name=f"pst{mm}_{r}")
                            nc.tensor.transpose(pst[:, :],
                                                src_t[:, r, mm, 0:D],
                                                ident_sb[:, :])
                            nc.vector.tensor_copy(dst_t[:, r, mm, :],
                                                  pst[:, :])

            ph1t.__exit__(None, None, None)
            ph1.__exit__(None, None, None)

            # ---- phase 2/3 PSUM pools
            ph2a = tc.tile_pool(name="psqk", bufs=3, space="PSUM")
            psqkp = ph2a.__enter__()
            ph2b = tc.tile_pool(name="psmall", bufs=5, space="PSUM")
            psmallp = ph2b.__enter__()

            # ---- phase 2+3 interleaved by row halves
            def phase2(r):
                qdc_f = qdc[:, r, :, :].rearrange("p a b -> p (a b)")
                aT = rowp.tile([128, NBLK, 520], dt.bfloat16, tag="aT",
                               name=f"aT{r}")
                for j in range(NBLK):
                    psA = psqkp.tile([128, 512], dt.float32, tag="psqk")
                    psB = psmallp.tile([128, 8], dt.float32, tag="psmall")
                    nc.tensor.matmul(psA[:, :], lhsT=kdc[:, r, j, :],
                                     rhs=qdc_f[:, 0:512], start=True, stop=True)
                    nc.tensor.matmul(psB[:, :], lhsT=kdc[:, r, j, :],
                                     rhs=qdc_f[:, 512:520], start=True, stop=True)
                    nc.scalar.activation(aT[:, j, 0:512], psA[:, :], Exp,
                                         bias=ln_sb[:, j:j + 1])
                    nc.scalar.activation(aT[:, j, 512:520], psB[:, :], Exp,
                                         bias=ln_sb[:, j:j + 1])

                for i in range(NBLK):
                    M = 128 if i < NBLK - 1 else (C - 128 * (NBLK - 1))
                    psv = psmallp.tile([M, D + 4], dt.float32, tag="psmall")
                    for j in range(NBLK):
                        nc.tensor.matmul(
                            psv[:, 0:D + 2],
                            lhsT=aT[:, j, 128 * i:128 * i + M],
                            rhs=v_aug[:, r, j, 0:D + 2],
                            start=(j == 0), stop=(j == NBLK - 1),
                        )
                    rv = smallp.tile([M, 1], dt.float32, tag="rv")
                    nc.vector.reciprocal(rv[:, :], psv[:, D:D + 1])
                    if M < 128:
                        nc.vector.memset(v_nrm[:, r, i, :], 0.0)
                    nc.vector.tensor_scalar_mul(v_nrm[0:M, r, i, :],
                                                psv[:, 0:D], rv[:, :])
                    nc.vector.tensor_mul(a0_all[0:M, i, r:r + 1],
                                         psv[:, D + 1:D + 2], rv[:, :])

            SCTOK = 1024
            sc_ops = (TT + SCTOK - 1) // SCTOK
            sc_tok = [min(SCTOK, TT - c * SCTOK) for c in range(sc_ops)]

            def phase3(half, rlo, nr, ext):
                hw_ = nr * D
                for c in range(sc_ops):
                    ntok = sc_tok[c]
                    osort = osortp.tile([128, SCTOK // 128, hw_], dt.bfloat16,
                                        tag=f"osort{half}",
                                        name=f"osort{half}_{c}")
                    for t_loc in range(ntok // 128):
                        t = c * (SCTOK // 128) + t_loc
                        pso = psmallp.tile([128, hw_], dt.float32,
                                           tag="psmall", name=f"pso{half}_{t}")
                        nc.tensor.matmul(
                            pso[:, :],
                            lhsT=PT_sb[:, bass.ts(t, 128)],
                            rhs=v_nrm[:, rlo:rlo + nr, BLK[t], :],
                            start=True, stop=True)
                        if t_loc % 2:
                            nc.scalar.copy(osort[:, t_loc, :], pso[:, :])
                        else:
                            nc.vector.tensor_copy(osort[:, t_loc, :], pso[:, :])
                    nc.gpsimd.dma_scatter_add(
                        ext[:, :],
                        osort[:, 0:ntok // 128, :],
                        is_sb[:, c * (SCTOK // 16):
                              c * (SCTOK // 16) + ntok // 16],
                        ntok,
                        ntok,
                        hw_,
                        single_packet=False,
                        queue_num=c % 2,
                    )

            for gi, (rlo, nr) in enumerate(groups):
                for r in range(rlo, rlo + nr):
                    phase2(r)
                phase3(gi, rlo, nr, out_exts[gi])
            nc.sync.dma_start(a0_ext[:, :],
                              a0_all[:, :, :].rearrange("p a b -> p (a b)"))
            ph2b.__exit__(None, None, None)
            ph2a.__exit__(None, None, None)

    return nc


# ------------------------------------------------------------------- runner

_CACHE = {}


def _ensure_ntff_hook():
    import types
    if "antenv.axon_hooks" in sys.modules:
        return
    m = types.ModuleType("antenv.axon_hooks")
    m._hook = None
    m.set_axon_ntff_profile_hook = lambda h: setattr(m, "_hook", h)
    m.get_axon_ntff_profile_hook = lambda: m._hook
    sys.modules["antenv.axon_hooks"] = m
    try:
        import antenv
        antenv.axon_hooks = m
    except Exception:
        pass
    try:
        from trn_agent_boot.trn_boot import _ntff_profile_via_ctypes
        hook = _ntff_profile_via_ctypes("/opt/axon/libaxon_pjrt.so")
        if hook is not None:
            m._hook = hook
    except Exception:
        pass


def make_in_maps(queries, keys, values, meta, n_rows=ROWS):
    bf16 = _bf16()
    in_maps = []
    for i in range(NCORES):
        cm = meta.cores[i]
        # token-major pack: qkv8[n, x*n_rows + r, :] = X_x[i + 8r, n, :]
        qkv = np.empty((NSEQ, 3, n_rows, D), dtype=bf16)
        for x, src in enumerate((queries, keys, values)):
            rows = src[i::NCORES][:n_rows]          # [n_rows, NSEQ, D]
            qkv[:, x, :, :] = rows.transpose(1, 0, 2).astype(bf16)
        in_maps.append({
            "qkv": qkv.reshape(NSEQ, 3 * n_rows * D),
            "P": cm.P_sb,
            "PT": cm.PT_sb,
            "idxg": cm.idx_g,
            "idxs": cm.idx_s,
            "wcol": cm.wcol,
            "lncnt": cm.lncnt,
        })
    return in_maps


def run_cores(queries, keys, values, clusters, trace=False, n_rows=ROWS):
    _ensure_ntff_hook()
    from concourse.bass_utils import run_bass_kernel_spmd

    meta = build_meta(np.asarray(clusters))
    key = (meta.T_m, n_rows)
    if key not in _CACHE:
        nc = build_nc(meta, n_rows)
        nc.finalize()
        _CACHE[key] = nc
    nc = _CACHE[key]
    in_maps = make_in_maps(np.asarray(queries), np.asarray(keys),
                           np.asarray(values), meta, n_rows)
    res = run_bass_kernel_spmd(nc, in_maps, core_ids=list(range(NCORES)),
                               trace=trace)
    return res, meta


def unshard(res, n_rows=ROWS):
    out = np.empty((B, NSEQ, D), np.float32)
    a0 = np.empty((B, C), np.float32)
    for i in range(NCORES):
        r = res.results[i]
        for gi, (rlo, nr) in enumerate(out_groups(n_rows)):
            og = np.asarray(r[f"out{gi}"][:NSEQ], np.float32).reshape(
                NSEQ, nr, D)
            for rr in range(nr):
                out[i + NCORES * (rlo + rr)] = og[:, rr, :]
        a0T = r["a0T"].reshape(128, NBLK, n_rows)
        for rr in range(n_rows):
            a0[i + NCORES * rr] = a0T[:, :, rr].T.reshape(CPAD)[:C]
    return out, a0


def kernel(**inputs):
    queries = np.asarray(inputs["queries"], np.float32)
    keys = np.asarray(inputs["keys"], np.float32)
    values = np.asarray(inputs["values"], np.float32)
    clusters = np.asarray(inputs["clusters"], np.int32)
    res, _ = run_cores(queries, keys, values, clusters, trace=False)
    return unshard(res)


# revision 27
# speedup vs baseline: 1.1898x; 1.1810x over previous
"""AdaClusteringAttention Trainium2 kernel (8 NeuronCores, batch/head parallel).

Reference semantics (per batch*head row b, cluster row = clusters[b % 8]):
  q_c/k_c/v_c = per-cluster means (segment-sum * 1/count)      [C=513, D=128]
  qk = q_c @ k_c^T ; a = softmax(qk) * counts ; a /= rowsum    [C, C]
  v  = a @ v_c ; out[n] = v[cluster[n]] ; a0 = a[:, 0]

Device strategy per core (8 rows each, all sharing ONE cluster row):
  - host: stable-sort tokens by cluster; pad each 128-cluster block's token
    list to a multiple of 128 (padding uniform across cores => one SPMD graph)
  - host packs q/k/v for the core's 8 rows token-major into one bf16 tensor
    qkv8[n, (tensor,row,d)] so a single dma_gather descriptor moves 6KB per
    token (Q7 descriptor generation is the scarce resource)
  - segment sums = per-128-token-chunk matmuls against one-hot blocks
    (a chunk's tokens all fall inside one 128-cluster block); row pairs are
    adjacent in the gathered layout so matmuls run at N=256
  - cluster attention: qkT = k_c q_c^T, aT = exp(qkT + ln(count[e]))
    (count-weighted softmax; max-subtraction skipped, scale cancels)
  - v_out = aT^T @ [v_c | 1] gives numerator and rowsum together
  - out tokens via one-hot-transpose matmuls in sorted order into a
    row-grouped buffer, then dma_scatter_add (4KB/token) back to token
    order (outputs are zero-initialized; pads go to dump row NSEQ)
"""

import sys

import numpy as np

B0, H, NSEQ, D = 8, 8, 4096, 128
B = B0 * H
C = 513
NBLK = 5            # ceil(C/128) cluster blocks
CPAD = NBLK * 128   # 640
NCORES = 8
ROWS = B // NCORES  # 8 rows per core
LN_NEG = -88.0      # exp(-88) == 0 in f32/bf16
OPTOK = 512         # tokens per gather/scatter op (SBUF footprint knob)


def _bf16():
    import ml_dtypes
    return ml_dtypes.bfloat16


# ----------------------------------------------------------------- host meta

class Meta:
    pass


def build_meta(clusters: np.ndarray) -> Meta:
    m = Meta()
    assert clusters.shape == (B0, NSEQ)
    counts = np.zeros((B0, CPAD), np.int64)
    for i in range(B0):
        counts[i, :C] = np.bincount(clusters[i], minlength=C)
    blk_tok = counts.reshape(B0, NBLK, 128).sum(-1)
    T_m = np.maximum(128, (np.ceil(blk_tok.max(0) / 128) * 128).astype(np.int64))
    m.T_m = tuple(int(x) for x in T_m)
    m.TT = int(T_m.sum())
    m.Tc = m.TT // 128
    offs = np.concatenate([[0], np.cumsum(T_m)]).astype(np.int64)
    m.offs = offs
    blk_of_chunk = []
    for mm in range(NBLK):
        blk_of_chunk += [mm] * (m.T_m[mm] // 128)
    m.blk_of_chunk = tuple(blk_of_chunk)
    first, last = {}, {}
    for t, mm in enumerate(m.blk_of_chunk):
        first.setdefault(mm, t)
        last[mm] = t
    m.first_chunk = first
    m.last_chunk = last

    bf16 = _bf16()
    m.cores = []
    for i in range(B0):
        cm = Meta()
        cl = clusters[i].astype(np.int64)
        order = np.argsort(cl, kind="stable")
        sc = cl[order]
        sblk = sc // 128
        idx_g = np.zeros(m.TT, np.int64)           # gather pad -> token 0
        idx_s = np.full(m.TT, NSEQ, np.int64)      # scatter pad -> dump row
        P = np.zeros((m.TT, 128), np.float32)
        for mm in range(NBLK):
            lo = int(np.searchsorted(sblk, mm))
            hi = int(np.searchsorted(sblk, mm + 1))
            if hi == lo:
                continue
            dst = offs[mm] + np.arange(hi - lo)
            idx_g[dst] = order[lo:hi]
            idx_s[dst] = order[lo:hi]
            P[dst, sc[lo:hi] - 128 * mm] = 1.0
        cm.P_sb = np.ascontiguousarray(
            P.reshape(m.Tc, 128, 128).transpose(1, 0, 2).reshape(128, m.Tc * 128)
        ).astype(bf16)
        # natural-order one-hot: PTnat[p, mm*NSEQ + n] = 1 iff cl[n] == 128*mm+p
        ptn = np.zeros((128, NBLK, NSEQ), np.float32)
        ptn[cl % 128, cl // 128, np.arange(NSEQ)] = 1.0
        cm.PTnat_sb = np.ascontiguousarray(
            ptn.reshape(128, NBLK * NSEQ)).astype(bf16)
        cm.idx_g = np.ascontiguousarray(np.tile(
            idx_g.reshape(m.TT // 16, 16).T, (8, 1))).astype(np.int16)
        cnts = counts[i].astype(np.float64)
        w = np.where(cnts > 0, 1.0 / np.maximum(cnts, 1), 0.0)
        lnc = np.where(cnts > 0, np.log(np.maximum(cnts, 1)), LN_NEG)
        cm.wcol = np.ascontiguousarray(
            w.reshape(NBLK, 128).T).astype(np.float32)
        cm.lncnt = np.ascontiguousarray(
            lnc.reshape(NBLK, 128).T).astype(np.float32)
        m.cores.append(cm)
    return m


def out_groups(n_rows):
    HR = max(n_rows // 2, 1)
    groups = [(0, HR)]
    if n_rows > HR:
        groups.append((HR, n_rows - HR))
    return groups


# ------------------------------------------------------------- bass builder

def build_nc(meta: Meta, n_rows: int = ROWS):
    import concourse.bacc as bacc
    import concourse.mybir as mybir
    import concourse.tile as tile
    from concourse import bass
    from concourse.masks import make_identity

    dt = mybir.dt
    Tc, TT, BLK = meta.Tc, meta.TT, meta.blk_of_chunk
    EW = 3 * n_rows * D           # gathered row width (elems, bf16)
    OW = n_rows * D               # out8 row width (elems, f32)
    n_ops = (TT + OPTOK - 1) // OPTOK
    op_tok = [min(OPTOK, TT - c * OPTOK) for c in range(n_ops)]

    nc = bacc.Bacc("TRN2", target_bir_lowering=False, debug=False,
                   num_devices=NCORES, num_swdge_queues=2)

    qkv_ext = nc.dram_tensor("qkv", [NSEQ, EW], dt.bfloat16, kind="ExternalInput")
    P_ext = nc.dram_tensor("P", [128, Tc * 128], dt.bfloat16, kind="ExternalInput")
    PTn_ext = nc.dram_tensor("PTnat", [128, NBLK * NSEQ], dt.bfloat16,
                             kind="ExternalInput")
    ig_ext = nc.dram_tensor("idxg", [128, TT // 16], dt.int16, kind="ExternalInput")
    w_ext = nc.dram_tensor("wcol", [128, NBLK], dt.float32, kind="ExternalInput")
    ln_ext = nc.dram_tensor("lncnt", [128, NBLK], dt.float32, kind="ExternalInput")
    groups = out_groups(n_rows)
    out_exts = [
        nc.dram_tensor(f"out{gi}", [NSEQ, nr * D], dt.bfloat16,
                       kind="ExternalOutput")
        for gi, (rlo, nr) in enumerate(groups)
    ]
    a0_ext = nc.dram_tensor("a0T", [128, NBLK * n_rows], dt.float32,
                            kind="ExternalOutput")

    Exp = mybir.ActivationFunctionType.Exp
    QW = min(4, n_rows)           # rows per segsum matmul (one PSUM bank)
    nquad = n_rows // QW

    with tile.TileContext(nc) as tc:
        with (
            tc.tile_pool(name="const", bufs=1) as constp,
            tc.tile_pool(name="gath", bufs=2) as gathp,
            tc.tile_pool(name="rowbuf", bufs=3) as rowp,
            tc.tile_pool(name="persist", bufs=1) as perp,
            tc.tile_pool(name="osort", bufs=3) as osortp,
            tc.tile_pool(name="small", bufs=4) as smallp,
        ):
            # ---- constants
            P_sb = constp.tile([128, Tc * 128], dt.bfloat16)
            PTn_sb = constp.tile([128, NBLK * NSEQ], dt.bfloat16)
            ig_sb = constp.tile([128, TT // 16], dt.int16)
            w_sb = constp.tile([128, NBLK], dt.float32)
            ln_sb = constp.tile([128, NBLK], dt.float32)
            ident_sb = constp.tile([128, 128], dt.bfloat16)
            nc.sync.dma_start(ig_sb[:, :], ig_ext[:, :])
            nc.sync.dma_start(w_sb[:, :], w_ext[:, :])
            nc.sync.dma_start(ln_sb[:, :], ln_ext[:, :])
            nc.sync.dma_start(P_sb[:, :], P_ext[:, :])
            nc.sync.dma_start(PTn_sb[:, :], PTn_ext[:, :])
            make_identity(nc, ident_sb[:, :])

            # ---- persistent per-row results
            qc_cd = perp.tile([128, n_rows, NBLK, D], dt.bfloat16)
            kc_cd = perp.tile([128, n_rows, NBLK, D], dt.bfloat16)
            v_aug = perp.tile([128, n_rows, NBLK, D + 4], dt.bfloat16)
            v_nrm = perp.tile([128, n_rows, NBLK, D], dt.bfloat16)
            a0_all = perp.tile([128, NBLK, n_rows], dt.float32)
            qdc = perp.tile([128, n_rows, NBLK, 128], dt.bfloat16)
            kdc = perp.tile([128, n_rows, NBLK, 128], dt.bfloat16)
            nc.vector.memset(a0_all[:, :, :], 0.0)
            nc.vector.memset(v_aug[:, :, :, D:D + 1], 1.0)
            # e0 column: picks out aT[0, :] inside the av matmul (e-chunk 0)
            nc.vector.memset(v_aug[:, :, :, D + 1:D + 2], 0.0)
            nc.vector.memset(v_aug[0:1, :, 0:1, D + 1:D + 2], 1.0)

            # ---- phase 1: gather + segment sums for all rows at once
            ph1 = tc.tile_pool(name="pscd", bufs=1, space="PSUM")
            pscdp = ph1.__enter__()
            ph1t = tc.tile_pool(name="pst", bufs=2, space="PSUM")
            pstp = ph1t.__enter__()
            ps_cd = {}
            for x in range(3):
                ps_cd[x] = pscdp.tile([128, n_rows * D], dt.float32,
                                      tag=f"cd{x}", name=f"ps_cd{x}")
            gtiles = []
            for c in range(n_ops):
                g = gathp.tile([128, OPTOK // 128, EW], dt.bfloat16, tag="gath")
                nc.gpsimd.dma_gather(
                    out_ap=g[:, 0:op_tok[c] // 128, :],
                    in_ap=qkv_ext[:, :],
                    idxs_ap=ig_sb[:, c * (OPTOK // 16):
                                  c * (OPTOK // 16) + op_tok[c] // 16],
                    num_idxs=op_tok[c],
                    num_idxs_reg=op_tok[c],
                    elem_size=EW,
                    single_packet=False,
                    queue_num=c % 2,
                )
                gtiles.append(g)

            for t in range(Tc):
                mm = BLK[t]
                c, t_loc = t // (OPTOK // 128), t % (OPTOK // 128)
                g = gtiles[c]
                for x in range(3):
                    for q in range(nquad):
                        nc.tensor.matmul(
                            ps_cd[x][:, bass.ts(q, QW * D)],
                            lhsT=P_sb[:, bass.ts(t, 128)],
                            rhs=g[:, t_loc,
                                  (x * n_rows + QW * q) * D:
                                  (x * n_rows + QW * (q + 1)) * D],
                            start=(t == meta.first_chunk[mm]),
                            stop=(t == meta.last_chunk[mm]),
                        )
                if t == meta.last_chunk[mm]:
                    for x, dst in ((0, qc_cd), (1, kc_cd), (2, v_aug)):
                        nc.vector.tensor_scalar_mul(
                            dst[:, :, mm, 0:D], ps_cd[x][:, :],
                            w_sb[:, mm:mm + 1])
                    for src_t, dst_t in ((qc_cd, qdc), (kc_cd, kdc)):
                        for r in range(n_rows):
                            pst = pstp.tile([128, 128], dt.bfloat16,
                                            tag="pst", name=f"pst{mm}_{r}")
                            nc.tensor.transpose(pst[:, :],
                                                src_t[:, r, mm, 0:D],
                                                ident_sb[:, :])
                            nc.vector.tensor_copy(dst_t[:, r, mm, :],
                                                  pst[:, :])

            ph1t.__exit__(None, None, None)
            ph1.__exit__(None, None, None)

            # ---- phase 2/3 PSUM pools
            ph2a = tc.tile_pool(name="psqk", bufs=3, space="PSUM")
            psqkp = ph2a.__enter__()
            ph2b = tc.tile_pool(name="psmall", bufs=5, space="PSUM")
            psmallp = ph2b.__enter__()

            # ---- phase 2+3 interleaved by row halves
            def phase2(r):
                qdc_f = qdc[:, r, :, :].rearrange("p a b -> p (a b)")
                aT = rowp.tile([128, NBLK, 520], dt.bfloat16, tag="aT",
                               name=f"aT{r}")
                for j in range(NBLK):
                    psA = psqkp.tile([128, 512], dt.float32, tag="psqk")
                    psB = psmallp.tile([128, 8], dt.float32, tag="psmall")
                    nc.tensor.matmul(psA[:, :], lhsT=kdc[:, r, j, :],
                                     rhs=qdc_f[:, 0:512], start=True, stop=True)
                    nc.tensor.matmul(psB[:, :], lhsT=kdc[:, r, j, :],
                                     rhs=qdc_f[:, 512:520], start=True, stop=True)
                    nc.scalar.activation(aT[:, j, 0:512], psA[:, :], Exp,
                                         bias=ln_sb[:, j:j + 1])
                    nc.scalar.activation(aT[:, j, 512:520], psB[:, :], Exp,
                                         bias=ln_sb[:, j:j + 1])

                for i in range(NBLK):
                    M = 128 if i < NBLK - 1 else (C - 128 * (NBLK - 1))
                    psv = psmallp.tile([M, D + 4], dt.float32, tag="psmall")
                    for j in range(NBLK):
                        nc.tensor.matmul(
                            psv[:, 0:D + 2],
                            lhsT=aT[:, j, 128 * i:128 * i + M],
                            rhs=v_aug[:, r, j, 0:D + 2],
                            start=(j == 0), stop=(j == NBLK - 1),
                        )
                    rv = smallp.tile([M, 1], dt.float32, tag="rv")
                    nc.vector.reciprocal(rv[:, :], psv[:, D:D + 1])
                    if M < 128:
                        nc.vector.memset(v_nrm[:, r, i, :], 0.0)
                    nc.vector.tensor_scalar_mul(v_nrm[0:M, r, i, :],
                                                psv[:, 0:D], rv[:, :])
                    nc.vector.tensor_mul(a0_all[0:M, i, r:r + 1],
                                         psv[:, D + 1:D + 2], rv[:, :])

            GT = 4                       # natural 128-token tiles per out DMA
            n3grp = NSEQ // (128 * GT)   # 8 groups per half

            def phase3_grp(half, rlo, nr, ext, c):
                hw_ = nr * D
                osn = osortp.tile([128, GT, hw_], dt.bfloat16,
                                  tag="osn", name=f"osn{half}_{c}")
                for t_loc in range(GT):
                    t = c * GT + t_loc
                    pso = psmallp.tile([128, hw_], dt.float32,
                                       tag="psmall", name=f"pso{half}_{t}")
                    for mm in range(NBLK):
                        nc.tensor.matmul(
                            pso[:, :],
                            lhsT=PTn_sb[:, mm * NSEQ + 128 * t:
                                        mm * NSEQ + 128 * (t + 1)],
                            rhs=v_nrm[:, rlo:rlo + nr, mm, :],
                            start=(mm == 0), stop=(mm == NBLK - 1))
                    if t_loc % 2:
                        nc.scalar.copy(osn[:, t_loc, :], pso[:, :])
                    else:
                        nc.vector.tensor_copy(osn[:, t_loc, :], pso[:, :])
                nc.sync.dma_start(
                    ext[128 * GT * c:128 * GT * (c + 1), :].rearrange(
                        "(a p) b -> p a b", p=128),
                    osn[:, :, :])

            for gi, (rlo, nr) in enumerate(groups):
                for r in range(rlo, rlo + nr):
                    phase2(r)
                for c in range(n3grp):
                    phase3_grp(gi, rlo, nr, out_exts[gi], c)
            nc.sync.dma_start(a0_ext[:, :],
                              a0_all[:, :, :].rearrange("p a b -> p (a b)"))
            ph2b.__exit__(None, None, None)
            ph2a.__exit__(None, None, None)

    return nc


# ------------------------------------------------------------------- runner

_CACHE = {}


def _ensure_ntff_hook():
    import types
    if "antenv.axon_hooks" in sys.modules:
        return
    m = types.ModuleType("antenv.axon_hooks")
    m._hook = None
    m.set_axon_ntff_profile_hook = lambda h: setattr(m, "_hook", h)
    m.get_axon_ntff_profile_hook = lambda: m._hook
    sys.modules["antenv.axon_hooks"] = m
    try:
        import antenv
        antenv.axon_hooks = m
    except Exception:
        pass
    try:
        from trn_agent_boot.trn_boot import _ntff_profile_via_ctypes
        hook = _ntff_profile_via_ctypes("/opt/axon/libaxon_pjrt.so")
        if hook is not None:
            m._hook = hook
    except Exception:
        pass


def make_in_maps(queries, keys, values, meta, n_rows=ROWS):
    bf16 = _bf16()
    in_maps = []
    for i in range(NCORES):
        cm = meta.cores[i]
        # token-major pack: qkv8[n, x*n_rows + r, :] = X_x[i + 8r, n, :]
        qkv = np.empty((NSEQ, 3, n_rows, D), dtype=bf16)
        for x, src in enumerate((queries, keys, values)):
            rows = src[i::NCORES][:n_rows]          # [n_rows, NSEQ, D]
            qkv[:, x, :, :] = rows.transpose(1, 0, 2).astype(bf16)
        in_maps.append({
            "qkv": qkv.reshape(NSEQ, 3 * n_rows * D),
            "P": cm.P_sb,
            "PTnat": cm.PTnat_sb,
            "idxg": cm.idx_g,
            "wcol": cm.wcol,
            "lncnt": cm.lncnt,
        })
    return in_maps


def run_cores(queries, keys, values, clusters, trace=False, n_rows=ROWS):
    _ensure_ntff_hook()
    from concourse.bass_utils import run_bass_kernel_spmd

    meta = build_meta(np.asarray(clusters))
    key = (meta.T_m, n_rows)
    if key not in _CACHE:
        nc = build_nc(meta, n_rows)
        nc.finalize()
        _CACHE[key] = nc
    nc = _CACHE[key]
    in_maps = make_in_maps(np.asarray(queries), np.asarray(keys),
                           np.asarray(values), meta, n_rows)
    res = run_bass_kernel_spmd(nc, in_maps, core_ids=list(range(NCORES)),
                               trace=trace)
    return res, meta


def unshard(res, n_rows=ROWS):
    out = np.empty((B, NSEQ, D), np.float32)
    a0 = np.empty((B, C), np.float32)
    for i in range(NCORES):
        r = res.results[i]
        for gi, (rlo, nr) in enumerate(out_groups(n_rows)):
            og = np.asarray(r[f"out{gi}"], np.float32).reshape(NSEQ, nr, D)
            for rr in range(nr):
                out[i + NCORES * (rlo + rr)] = og[:, rr, :]
        a0T = r["a0T"].reshape(128, NBLK, n_rows)
        for rr in range(n_rows):
            a0[i + NCORES * rr] = a0T[:, :, rr].T.reshape(CPAD)[:C]
    return out, a0


def kernel(**inputs):
    queries = np.asarray(inputs["queries"], np.float32)
    keys = np.asarray(inputs["keys"], np.float32)
    values = np.asarray(inputs["values"], np.float32)
    clusters = np.asarray(inputs["clusters"], np.int32)
    res, _ = run_cores(queries, keys, values, clusters, trace=False)
    return unshard(res)
